# revision 19
# baseline (speedup 1.0000x reference)
"""Trainium2 Bass kernel for the nn_Decoder LSTM-decoder problem.

Problem (from the reference):
    64-step LSTM(E=64 -> H=128) decoder over B=16384 independent agents.
    Per step: gates = x@W_ih.T + h@W_hh.T + b; standard LSTM cell;
    rel = h@W_pos.T + b_pos (output per step); x_next = rel@W_emb.T + b_emb.
    Returns (pred_traj [64,B,2], h_final [1,B,H]).

Key algebraic collapse: the carried `lp` is never output, and x_{t+1} is a
linear function of h_t, so for t>=1
    gates_t = W_eff @ h_{t-1} + b_eff,   W_eff = W_ih@(W_emb@W_pos) + W_hh
and for t=0
    gates_0 = W0 @ last_pos_rel + W_hh @ h_0 + b_0,  W0 = W_ih@W_emb.
All weight folding is done on the host in float64 and cast to float32.

Device layout: features on partitions (H=128 exactly), batch on the free
dimension -> no transposes anywhere on device. Data-parallel over batch
across 8 NeuronCores (2048 agents per core).

Per core, per step t (4 batch chunks of 512, all matmuls in float32r — the
single-pass fp32 PE mode, ~1.5e-4 per-matmul rel err vs 4x slower exact fp32):
    PE : 4 matmuls (K=128, M=128, N=512), g-gate first, into split PSUM:
         i/f/o -> double-buffered 3-bank tile, g -> 1 bank (so next chunk's
         matmuls overlap this chunk's sigmoid read; 6+1+1 banks = all 8)
    ACT: sigmoid over [128, 3*512] (i,f,o) ; tanh over [128,512] (g)
    POOL: t1 = sig_i*tanh_g ; t2 = sig_f*c   (gpsimd tensor_tensor never
         touches the DVE-shared SBUF port pair -> runs truly parallel)
    DVE: c' = t1 + t2
    ACT: tanh(c') at 1024 width (once per chunk pair)
    DVE: h' = sig_o * tanh_c  (written as float32r for the next matmuls)
    PE : rel^T per 128-batch subchunk: (h' slice [128K,128M])^T @ W_pos^T[128,2]
         -> PSUM [128 batch, 2] slices packed into one bank [128, 16, 2]
    DVE: single [128,32] PSUM->SBUF copy per step; contiguous DMA to HBM.
b_pos is added to the trajectory on the host (it is outside the recurrence).

Numerics notes: tanh(g) must be a real Tanh (not 2*sigmoid(2g)-1) because the
system decays to ~1e-17 and the sigmoid form cancels catastrophically near
0.5.  c stays fp32.  float32r's in-memory format is not IEEE fp32, so the
final h is converted by a DVE copy before the output DMA.
"""

import os
import sys

import numpy as np

SEQ = 64
B = 16384
E = 64
H = 128
NCORES = 8
BSH = B // NCORES          # 2048 agents per core
CHUNK = 512                # batch columns per PSUM gate tile
NCHUNK = BSH // CHUNK      # 4
SUBS = BSH // H            # 16 rel subchunks per step

_bass = None
_mybir = None
_tile = None
_run_spmd = None

LAST_RESULTS = None        # BassKernelResults of the most recent run


def _ensure_imports():
    global _bass, _mybir, _tile, _run_spmd
    if _bass is not None:
        return
    try:
        import concourse.bass as bass
    except ImportError:
        for p in ("/opt/trn_rl_repo", os.path.expanduser("~/.axon_site/_ro/trn_rl_repo")):
            if os.path.isdir(p) and p not in sys.path:
                sys.path.insert(0, p)
        import concourse.bass as bass
    import concourse.bacc as bacc
    import concourse.mybir as mybir
    import concourse.tile as tile
    from concourse.bass_utils import run_bass_kernel_spmd
    _bass, _mybir, _tile, _run_spmd = bacc, mybir, tile, run_bass_kernel_spmd


_MODULE_CACHE = {}


def _build_module(has_beff, has_b0):
    """Build the single-core SPMD Bass module (same program on all 8 cores)."""
    _ensure_imports()
    bass, mybir, tile = _bass, _mybir, _tile
    f32 = mybir.dt.float32
    f32r = mybir.dt.float32r
    AF = mybir.ActivationFunctionType

    nc = bass.Bacc(None)

    w_eff_d = nc.dram_tensor("w_effT", [H, 4 * H], f32r, kind="ExternalInput")
    w_hh_d = nc.dram_tensor("w_hhT", [H, 4 * H], f32r, kind="ExternalInput")
    w0_d = nc.dram_tensor("w0T", [2, 4 * H], f32r, kind="ExternalInput")
    w_pos_d = nc.dram_tensor("w_posT", [H, 2], f32r, kind="ExternalInput")
    h0_d = nc.dram_tensor("h0T", [H, BSH], f32r, kind="ExternalInput")
    c0_d = nc.dram_tensor("c0T", [H, BSH], f32, kind="ExternalInput")
    lpr_d = nc.dram_tensor("lprT", [2, BSH], f32r, kind="ExternalInput")
    beff_d = nc.dram_tensor("beff", [1, 4 * H], f32r, kind="ExternalInput") if has_beff else None
    b0_d = nc.dram_tensor("b0", [1, 4 * H], f32r, kind="ExternalInput") if has_b0 else None

    traj_d = nc.dram_tensor("traj", [SEQ, H, SUBS, 2], f32, kind="ExternalOutput")
    hf_d = nc.dram_tensor("hf", [H, BSH], f32, kind="ExternalOutput")

    with tile.TileContext(nc) as tc:
        with (
            tc.tile_pool(name="const", bufs=1) as constp,
            tc.tile_pool(name="state", bufs=1) as statep,
            tc.tile_pool(name="gact", bufs=3) as gactp,
            tc.tile_pool(name="tmp", bufs=4) as tmpp,
            tc.tile_pool(name="relsb", bufs=3) as relsbp,
            tc.tile_pool(name="psum", bufs=1, space="PSUM") as psump,
        ):
            w_eff = constp.tile([H, 4 * H], f32r, tag="w_eff")
            nc.sync.dma_start(w_eff[:], w_eff_d[:])
            w_hh = constp.tile([H, 4 * H], f32r, tag="w_hh")
            nc.sync.dma_start(w_hh[:], w_hh_d[:])
            w0 = constp.tile([2, 4 * H], f32r, tag="w0")
            nc.sync.dma_start(w0[:], w0_d[:])
            w_pos = constp.tile([H, 2], f32r, tag="w_pos")
            nc.sync.dma_start(w_pos[:], w_pos_d[:])
            lpr = constp.tile([2, BSH], f32r, tag="lpr")
            nc.sync.dma_start(lpr[:], lpr_d[:])

            ones = None
            beff_sb = None
            b0_sb = None
            if has_beff or has_b0:
                ones = constp.tile([1, CHUNK], f32r, tag="ones")
                nc.vector.memset(ones[:], 1.0)
            if has_beff:
                beff_sb = constp.tile([1, 4 * H], f32r, tag="beff")
                nc.sync.dma_start(beff_sb[:], beff_d[:])
            if has_b0:
                b0_sb = constp.tile([1, 4 * H], f32r, tag="b0")
                nc.sync.dma_start(b0_sb[:], b0_d[:])

            h_sb = [statep.tile([H, BSH], f32r, tag=f"h{i}", name=f"h{i}") for i in range(2)]
            c_sb = [statep.tile([H, BSH], f32, tag=f"c{i}", name=f"c{i}") for i in range(2)]
            # Collapse the many distinct DMA-queue semaphore waits into one
            # barrier; walrus rejects instructions with too many sync waits.
            # The big state loads go after it so their consumers take direct
            # (few-sem) waits instead of delaying the whole preamble.
            tc.strict_bb_all_engine_barrier()
            nc.sync.dma_start(h_sb[0][:], h0_d[:])
            nc.sync.dma_start(c_sb[0][:], c0_d[:])

            for t in range(SEQ):
                hp = h_sb[t % 2]
                hn = h_sb[(t + 1) % 2]
                cp = c_sb[t % 2]
                cn = c_sb[(t + 1) % 2]
                relp = psump.tile([H, SUBS, 2], f32, tag="rel", bufs=1, name=f"relp_{t}")
                tncs = []
                for ci in range(NCHUNK):
                    csl = slice(ci * CHUNK, (ci + 1) * CHUNK)
                    # i/f/o gates in a double-buffered 3-bank PSUM tile so the
                    # next chunk's matmuls overlap this chunk's sigmoid read;
                    # g gate in its own single-buffered bank.
                    gpi = psump.tile([H, 3, CHUNK], f32, tag="gpi", bufs=2, name=f"gpi_{t}_{ci}")
                    gpg = psump.tile([H, CHUNK], f32, tag="gpg", bufs=1, name=f"gpg_{t}_{ci}")
                    for g in (3, 0, 1, 2):
                        gsl = slice(g * H, (g + 1) * H)
                        dst = gpi[:, g, :] if g < 3 else gpg[:]
                        if t == 0:
                            bias_mm = has_b0
                            nc.tensor.matmul(dst, w0[:, gsl], lpr[:, csl],
                                             start=True, stop=False)
                            nc.tensor.matmul(dst, w_hh[:, gsl], hp[:, csl],
                                             start=False, stop=not bias_mm)
                            if bias_mm:
                                nc.tensor.matmul(dst, b0_sb[:, gsl], ones[:],
                                                 start=False, stop=True)
                        else:
                            bias_mm = has_beff
                            nc.tensor.matmul(dst, w_eff[:, gsl], hp[:, csl],
                                             start=True, stop=not bias_mm)
                            if bias_mm:
                                nc.tensor.matmul(dst, beff_sb[:, gsl], ones[:],
                                                 start=False, stop=True)
                    # gates order: [i, f, o | g].  tanh(g) must be a real
                    # Tanh: the system decays to ~1e-17 magnitudes and
                    # 2*sigmoid(2g)-1 cancels catastrophically for small g.
                    sig = gactp.tile([H, 3, CHUNK], f32, tag="sig", name=f"sig_{t}_{ci}", bufs=6)
                    nc.scalar.activation(sig[:], gpi[:], AF.Sigmoid)
                    tg = gactp.tile([H, CHUNK], f32, tag="tg", name=f"tg_{t}_{ci}", bufs=4)
                    nc.scalar.activation(tg[:], gpg[:], AF.Tanh)
                    # the two gate products run on GpSimd (tensor_tensor never
                    # touches the shared SBUF port pair -> no DVE contention)
                    t2 = tmpp.tile([H, CHUNK], f32, tag="t2", name=f"t2_{t}_{ci}")
                    nc.gpsimd.tensor_mul(t2[:], sig[:, 1, :], cp[:, csl])
                    t1 = tmpp.tile([H, CHUNK], f32, tag="t1", name=f"t1_{t}_{ci}")
                    nc.gpsimd.tensor_mul(t1[:], sig[:, 0, :], tg[:])
                    nc.vector.tensor_add(cn[:, csl], t1[:], t2[:])
                    tncs.append((ci, sig))
                    if ci % 2 == 1:
                        hsl = slice((ci - 1) * CHUNK, (ci + 1) * CHUNK)
                        tnc = tmpp.tile([H, 2 * CHUNK], f32, tag="tnc", name=f"tnc_{t}_{ci}")
                        nc.scalar.activation(tnc[:], cn[:, hsl], AF.Tanh)
                        for cj, sg in tncs:
                            ksl = slice(cj * CHUNK, (cj + 1) * CHUNK)
                            off = (cj % 2) * CHUNK
                            nc.vector.tensor_mul(hn[:, ksl], sg[:, 2, :],
                                                 tnc[:, off:off + CHUNK])
                        tncs = []
                        # rel matmuls must be emitted after the h writes they
                        # read, or Tile binds them to the stale double-buffer
                        for sc in range((ci - 1) * (CHUNK // H), (ci + 1) * (CHUNK // H)):
                            bs = sc * H
                            nc.tensor.matmul(relp[:, sc, :], hn[:, bs:bs + H], w_pos[:],
                                             start=True, stop=True)
                rel_sb = relsbp.tile([H, SUBS, 2], f32, tag="rel_sb", name=f"relsb_{t}")
                nc.vector.tensor_copy(rel_sb[:], relp[:])
                nc.sync.dma_start(traj_d[t], rel_sb[:])
            # h lives in f32r whose in-memory format is not IEEE fp32;
            # convert once before shipping the final hidden state out.
            hf_sb = relsbp.tile([H, BSH], f32, tag="hf_sb", name="hf_sb")
            nc.vector.tensor_copy(hf_sb[:], h_sb[SEQ % 2][:])
            nc.sync.dma_start(hf_d[:], hf_sb[:])

    nc.finalize()
    return nc


def _get_module(has_beff, has_b0):
    key = (has_beff, has_b0)
    if key not in _MODULE_CACHE:
        _MODULE_CACHE[key] = _build_module(has_beff, has_b0)
    return _MODULE_CACHE[key]


# torch LSTM gate order is (i, f, g, o); device layout wants (i, f, o, g)
_GATE_PERM = np.concatenate([
    np.arange(0, H), np.arange(H, 2 * H), np.arange(3 * H, 4 * H),
    np.arange(2 * H, 3 * H),
])


def kernel(**inputs):
    global LAST_RESULTS
    _ensure_imports()

    inp = {k: np.asarray(v) for k, v in inputs.items()}

    W_ih = inp["W_ih"].astype(np.float64)
    W_hh = inp["W_hh"].astype(np.float64)
    W_emb = inp["W_emb"].astype(np.float64)
    W_pos = inp["W_pos"].astype(np.float64)
    b_ih = inp["b_ih"].astype(np.float64)
    b_hh = inp["b_hh"].astype(np.float64)
    b_emb = inp["b_emb"].astype(np.float64)
    b_pos = inp["b_pos"].astype(np.float64)

    W_eff = W_ih @ (W_emb @ W_pos) + W_hh                 # [4H, H]
    b_eff = W_ih @ (W_emb @ b_pos + b_emb) + b_ih + b_hh  # [4H]
    W0 = W_ih @ W_emb                                     # [4H, 2]
    b0 = W_ih @ b_emb + b_ih + b_hh                       # [4H]

    p = _GATE_PERM
    W_effT = np.ascontiguousarray(W_eff[p].T, np.float32)   # [H, 4H]
    W_hhT = np.ascontiguousarray(W_hh[p].T, np.float32)     # [H, 4H]
    W0T = np.ascontiguousarray(W0[p].T, np.float32)         # [2, 4H]
    W_posT = np.ascontiguousarray(W_pos.T, np.float32)      # [H, 2]
    beff_p = np.ascontiguousarray(b_eff[p], np.float32)[None]  # [1, 4H]
    b0_p = np.ascontiguousarray(b0[p], np.float32)[None]

    has_beff = bool(np.any(beff_p))
    has_b0 = bool(np.any(b0_p))

    h0T = np.ascontiguousarray(inp["hh"][0].T, np.float32)   # [H, B]
    c0T = np.ascontiguousarray(inp["ch"][0].T, np.float32)
    lprT = np.ascontiguousarray(inp["last_pos_rel"].T, np.float32)  # [2, B]

    nc = _get_module(has_beff, has_b0)

    in_maps = []
    for c in range(NCORES):
        cols = slice(c * BSH, (c + 1) * BSH)
        m = {
            "w_effT": W_effT,
            "w_hhT": W_hhT,
            "w0T": W0T,
            "w_posT": W_posT,
            "h0T": np.ascontiguousarray(h0T[:, cols]),
            "c0T": np.ascontiguousarray(c0T[:, cols]),
            "lprT": np.ascontiguousarray(lprT[:, cols]),
        }
        if has_beff:
            m["beff"] = beff_p
        if has_b0:
            m["b0"] = b0_p
        in_maps.append(m)

    trace = bool(int(os.environ.get("KBENCH_TRACE", "0")))
    tmpdir = os.environ.get("KBENCH_TMPDIR") or None
    res = _run_spmd(nc, in_maps, list(range(NCORES)), trace=trace, tmpdir=tmpdir)
    LAST_RESULTS = res

    traj_parts = []
    h_parts = []
    for c in range(NCORES):
        tr = np.asarray(res.results[c]["traj"])       # [SEQ, H, SUBS, 2]
        traj_parts.append(tr.transpose(0, 2, 1, 3).reshape(SEQ, BSH, 2))
        h_parts.append(np.asarray(res.results[c]["hf"]).T)  # [BSH, H]

    traj = np.concatenate(traj_parts, axis=1)
    traj = (traj + inp["b_pos"][None, None, :].astype(np.float32)).astype(np.float32)
    hF = np.concatenate(h_parts, axis=0)[None].astype(np.float32)
    return traj, hF


# revision 21
# speedup vs baseline: 1.0357x; 1.0357x over previous
"""Trainium2 Bass kernel for the nn_Decoder LSTM-decoder problem.

Problem (from the reference):
    64-step LSTM(E=64 -> H=128) decoder over B=16384 independent agents.
    Per step: gates = x@W_ih.T + h@W_hh.T + b; standard LSTM cell;
    rel = h@W_pos.T + b_pos (output per step); x_next = rel@W_emb.T + b_emb.
    Returns (pred_traj [64,B,2], h_final [1,B,H]).

Key algebraic collapse: the carried `lp` is never output, and x_{t+1} is a
linear function of h_t, so for t>=1
    gates_t = W_eff @ h_{t-1} + b_eff,   W_eff = W_ih@(W_emb@W_pos) + W_hh
and for t=0
    gates_0 = W0 @ last_pos_rel + W_hh @ h_0 + b_0,  W0 = W_ih@W_emb.
All weight folding is done on the host in float64 and cast to float32.

Device layout: features on partitions (H=128 exactly), batch on the free
dimension -> no transposes anywhere on device. Data-parallel over batch
across 8 NeuronCores (2048 agents per core).

Per core, per step t (4 batch chunks of 512, all matmuls in float32r — the
single-pass fp32 PE mode, ~1.5e-4 per-matmul rel err vs 4x slower exact fp32):
    PE : 4 matmuls (K=128, M=128, N=512), g-gate first, into split PSUM:
         i/f/o -> double-buffered 3-bank tile, g -> 1 bank (so next chunk's
         matmuls overlap this chunk's sigmoid read; 6+1+1 banks = all 8)
    ACT: sigmoid over [128, 3*512] (i,f,o) ; tanh over [128,512] (g)
    POOL: t1 = sig_i*tanh_g ; t2 = sig_f*c   (gpsimd tensor_tensor never
         touches the DVE-shared SBUF port pair -> runs truly parallel)
    DVE: c' = t1 + t2
    ACT: tanh(c') at 1024 width (once per chunk pair)
    DVE: h' = sig_o * tanh_c  (written as float32r for the next matmuls)
    PE : rel^T per 128-batch subchunk: (h' slice [128K,128M])^T @ W_pos^T[128,2]
         -> PSUM [128 batch, 2] slices packed into one bank [128, 16, 2]
    DVE: single [128,32] PSUM->SBUF copy per step; contiguous DMA to HBM.
b_pos is added to the trajectory on the host (it is outside the recurrence).

Numerics notes: tanh(g) must be a real Tanh (not 2*sigmoid(2g)-1) because the
system decays to ~1e-17 and the sigmoid form cancels catastrophically near
0.5.  c stays fp32.  float32r's in-memory format is not IEEE fp32, so the
final h is converted by a DVE copy before the output DMA.
"""

import os
import sys

import numpy as np

SEQ = 64
B = 16384
E = 64
H = 128
NCORES = 8
BSH = B // NCORES          # 2048 agents per core
CHUNK = 512                # batch columns per PSUM gate tile
NCHUNK = BSH // CHUNK      # 4
SUBS = BSH // H            # 16 rel subchunks per step

_bass = None
_mybir = None
_tile = None
_run_spmd = None

LAST_RESULTS = None        # BassKernelResults of the most recent run


def _ensure_imports():
    global _bass, _mybir, _tile, _run_spmd
    if _bass is not None:
        return
    try:
        import concourse.bass as bass
    except ImportError:
        for p in ("/opt/trn_rl_repo", os.path.expanduser("~/.axon_site/_ro/trn_rl_repo")):
            if os.path.isdir(p) and p not in sys.path:
                sys.path.insert(0, p)
        import concourse.bass as bass
    import concourse.bacc as bacc
    import concourse.mybir as mybir
    import concourse.tile as tile
    from concourse.bass_utils import run_bass_kernel_spmd
    _bass, _mybir, _tile, _run_spmd = bacc, mybir, tile, run_bass_kernel_spmd


_MODULE_CACHE = {}


def _build_module(has_beff, has_b0):
    """Build the single-core SPMD Bass module (same program on all 8 cores)."""
    _ensure_imports()
    bass, mybir, tile = _bass, _mybir, _tile
    f32 = mybir.dt.float32
    f32r = mybir.dt.float32r
    AF = mybir.ActivationFunctionType

    nc = bass.Bacc(None)

    w_eff_d = nc.dram_tensor("w_effT", [H, 4 * H], f32r, kind="ExternalInput")
    w_hh_d = nc.dram_tensor("w_hhT", [H, 4 * H], f32r, kind="ExternalInput")
    w0_d = nc.dram_tensor("w0T", [2, 4 * H], f32r, kind="ExternalInput")
    w_pos_d = nc.dram_tensor("w_posT", [H, 2], f32r, kind="ExternalInput")
    h0_d = nc.dram_tensor("h0T", [H, BSH], f32r, kind="ExternalInput")
    c0_d = nc.dram_tensor("c0T", [H, BSH], f32, kind="ExternalInput")
    lpr_d = nc.dram_tensor("lprT", [2, BSH], f32r, kind="ExternalInput")
    beff_d = nc.dram_tensor("beff", [1, 4 * H], f32r, kind="ExternalInput") if has_beff else None
    b0_d = nc.dram_tensor("b0", [1, 4 * H], f32r, kind="ExternalInput") if has_b0 else None

    traj_d = nc.dram_tensor("traj", [SEQ, H, SUBS, 2], f32, kind="ExternalOutput")
    hf_d = nc.dram_tensor("hf", [H, BSH], f32, kind="ExternalOutput")

    with tile.TileContext(nc) as tc:
        with (
            tc.tile_pool(name="const", bufs=1) as constp,
            tc.tile_pool(name="state", bufs=1) as statep,
            tc.tile_pool(name="gact", bufs=3) as gactp,
            tc.tile_pool(name="tmp", bufs=4) as tmpp,
            tc.tile_pool(name="relsb", bufs=3) as relsbp,
            tc.tile_pool(name="psum", bufs=1, space="PSUM") as psump,
        ):
            w_eff = constp.tile([H, 4 * H], f32r, tag="w_eff")
            nc.sync.dma_start(w_eff[:], w_eff_d[:])
            w_hh = constp.tile([H, 4 * H], f32r, tag="w_hh")
            nc.sync.dma_start(w_hh[:], w_hh_d[:])
            w0 = constp.tile([2, 4 * H], f32r, tag="w0")
            nc.sync.dma_start(w0[:], w0_d[:])
            w_pos = constp.tile([H, 2], f32r, tag="w_pos")
            nc.sync.dma_start(w_pos[:], w_pos_d[:])
            lpr = constp.tile([2, BSH], f32r, tag="lpr")
            nc.sync.dma_start(lpr[:], lpr_d[:])

            ones = None
            beff_sb = None
            b0_sb = None
            if has_beff or has_b0:
                ones = constp.tile([1, CHUNK], f32r, tag="ones")
                nc.vector.memset(ones[:], 1.0)
            if has_beff:
                beff_sb = constp.tile([1, 4 * H], f32r, tag="beff")
                nc.sync.dma_start(beff_sb[:], beff_d[:])
            if has_b0:
                b0_sb = constp.tile([1, 4 * H], f32r, tag="b0")
                nc.sync.dma_start(b0_sb[:], b0_d[:])

            h_sb = [statep.tile([H, BSH], f32r, tag=f"h{i}", name=f"h{i}") for i in range(2)]
            # c state interleaved with g-gate staging: plane (ci, 0) = c chunk,
            # plane (ci, 1) = tanh-staging for the g gate of the NEXT chunk, so
            # one 1024-wide Tanh covers [c'(ci) | g(ci+1)] in a single pass.
            c_sb = [statep.tile([H, NCHUNK, 2, CHUNK], f32, tag=f"c{i}", name=f"c{i}")
                    for i in range(2)]
            # Collapse the many distinct DMA-queue semaphore waits into one
            # barrier; walrus rejects instructions with too many sync waits.
            # The big state loads go after it so their consumers take direct
            # (few-sem) waits instead of delaying the whole preamble.
            tc.strict_bb_all_engine_barrier()
            nc.sync.dma_start(h_sb[0][:], h0_d[:])
            nc.sync.dma_start(c_sb[0][:, :, 0, :],
                              c0_d[:].rearrange("p (a b) -> p a b", b=CHUNK))

            relp_prev = None
            prev_sig3 = None
            for t in range(SEQ):
                hp = h_sb[t % 2]
                hn = h_sb[(t + 1) % 2]
                cp = c_sb[t % 2]
                cn = c_sb[(t + 1) % 2]
                relp = psump.tile([H, SUBS, 2], f32, tag="rel", bufs=1, name=f"relp_{t}")
                sig_list = [None] * NCHUNK
                for ci in range(NCHUNK):
                    csl = slice(ci * CHUNK, (ci + 1) * CHUNK)
                    # i/f/o gates in a double-buffered 3-bank PSUM tile so the
                    # next chunk's matmuls overlap this chunk's sigmoid read;
                    # g gate in its own single-buffered bank.
                    gpi = psump.tile([H, 3, CHUNK], f32, tag="gpi", bufs=2, name=f"gpi_{t}_{ci}")
                    gpg = psump.tile([H, CHUNK], f32, tag="gpg", bufs=1, name=f"gpg_{t}_{ci}")
                    for g in (3, 0, 1, 2):
                        gsl = slice(g * H, (g + 1) * H)
                        dst = gpi[:, g, :] if g < 3 else gpg[:]
                        if t == 0:
                            bias_mm = has_b0
                            nc.tensor.matmul(dst, w0[:, gsl], lpr[:, csl],
                                             start=True, stop=False)
                            nc.tensor.matmul(dst, w_hh[:, gsl], hp[:, csl],
                                             start=False, stop=not bias_mm)
                            if bias_mm:
                                nc.tensor.matmul(dst, b0_sb[:, gsl], ones[:],
                                                 start=False, stop=True)
                        else:
                            bias_mm = has_beff
                            nc.tensor.matmul(dst, w_eff[:, gsl], hp[:, csl],
                                             start=True, stop=not bias_mm)
                            if bias_mm:
                                nc.tensor.matmul(dst, beff_sb[:, gsl], ones[:],
                                                 start=False, stop=True)
                    # tanh(g) must be a real Tanh: the system decays to ~1e-17
                    # magnitudes and 2*sigmoid(2g)-1 cancels catastrophically.
                    sig = gactp.tile([H, 3, CHUNK], f32, tag="sig", name=f"sig_{t}_{ci}", bufs=6)
                    nc.scalar.activation(sig[:], gpi[:], AF.Sigmoid)
                    sig_list[ci] = sig
                    # stage g next to the previous chunk's c' and tanh both at
                    # once; chunk 0 pairs with c'(3) of the previous step.
                    pairT, pslot = (cn, ci - 1) if ci >= 1 else (cp, NCHUNK - 1)
                    nc.vector.tensor_copy(pairT[:, pslot, 1, :], gpg[:])
                    tout = gactp.tile([H, 2, CHUNK], f32, tag="tout", name=f"tout_{t}_{ci}", bufs=4)
                    nc.scalar.activation(tout[:], pairT[:, pslot, :, :], AF.Tanh)
                    # finish the previous chunk's h = sig_o * tanh(c') and its
                    # rel matmuls (must be emitted after the h write they read)
                    if ci >= 1:
                        hc, htile, sg, rtgt = ci - 1, hn, sig_list[ci - 1], relp
                    else:
                        hc, htile, sg, rtgt = NCHUNK - 1, hp, prev_sig3, relp_prev
                    if ci >= 1 or t > 0:
                        hsl = slice(hc * CHUNK, (hc + 1) * CHUNK)
                        nc.vector.tensor_mul(htile[:, hsl], sg[:, 2, :], tout[:, 0, :])
                        for s4 in range(CHUNK // H):
                            sc = hc * (CHUNK // H) + s4
                            bs = hc * CHUNK + s4 * H
                            nc.tensor.matmul(rtgt[:, sc, :], htile[:, bs:bs + H],
                                             w_pos[:], start=True, stop=True)
                        if ci == 0:
                            rel_sb = relsbp.tile([H, SUBS, 2], f32, tag="rel_sb",
                                                 name=f"relsb_{t - 1}")
                            nc.vector.tensor_copy(rel_sb[:], relp_prev[:])
                            nc.sync.dma_start(traj_d[t - 1], rel_sb[:])
                    # gpsimd products never touch the DVE-shared SBUF port pair
                    t2 = tmpp.tile([H, CHUNK], f32, tag="t2", name=f"t2_{t}_{ci}")
                    nc.gpsimd.tensor_mul(t2[:], sig[:, 1, :], cp[:, ci, 0, :])
                    t1 = tmpp.tile([H, CHUNK], f32, tag="t1", name=f"t1_{t}_{ci}")
                    nc.gpsimd.tensor_mul(t1[:], sig[:, 0, :], tout[:, 1, :])
                    nc.gpsimd.tensor_add(cn[:, ci, 0, :], t1[:], t2[:])
                prev_sig3 = sig_list[NCHUNK - 1]
                relp_prev = relp

            # epilogue: last chunk of the last step has no following tanh pair
            cl = c_sb[SEQ % 2]
            hl = h_sb[SEQ % 2]
            tnc3 = tmpp.tile([H, CHUNK], f32, tag="tnc3", name="tnc3")
            nc.scalar.activation(tnc3[:], cl[:, NCHUNK - 1, 0, :], AF.Tanh)
            lsl = slice((NCHUNK - 1) * CHUNK, NCHUNK * CHUNK)
            nc.vector.tensor_mul(hl[:, lsl], prev_sig3[:, 2, :], tnc3[:])
            for s4 in range(CHUNK // H):
                sc = (NCHUNK - 1) * (CHUNK // H) + s4
                bs = (NCHUNK - 1) * CHUNK + s4 * H
                nc.tensor.matmul(relp_prev[:, sc, :], hl[:, bs:bs + H], w_pos[:],
                                 start=True, stop=True)
            rel_sb = relsbp.tile([H, SUBS, 2], f32, tag="rel_sb", name=f"relsb_{SEQ - 1}")
            nc.vector.tensor_copy(rel_sb[:], relp_prev[:])
            nc.sync.dma_start(traj_d[SEQ - 1], rel_sb[:])
            # h lives in f32r whose in-memory format is not IEEE fp32;
            # convert once before shipping the final hidden state out.
            hf_sb = relsbp.tile([H, BSH], f32, tag="hf_sb", name="hf_sb")
            nc.vector.tensor_copy(hf_sb[:], hl[:])
            nc.sync.dma_start(hf_d[:], hf_sb[:])

    nc.finalize()
    return nc


def _get_module(has_beff, has_b0):
    key = (has_beff, has_b0)
    if key not in _MODULE_CACHE:
        _MODULE_CACHE[key] = _build_module(has_beff, has_b0)
    return _MODULE_CACHE[key]


# torch LSTM gate order is (i, f, g, o); device layout wants (i, f, o, g)
_GATE_PERM = np.concatenate([
    np.arange(0, H), np.arange(H, 2 * H), np.arange(3 * H, 4 * H),
    np.arange(2 * H, 3 * H),
])


def kernel(**inputs):
    global LAST_RESULTS
    _ensure_imports()

    inp = {k: np.asarray(v) for k, v in inputs.items()}

    W_ih = inp["W_ih"].astype(np.float64)
    W_hh = inp["W_hh"].astype(np.float64)
    W_emb = inp["W_emb"].astype(np.float64)
    W_pos = inp["W_pos"].astype(np.float64)
    b_ih = inp["b_ih"].astype(np.float64)
    b_hh = inp["b_hh"].astype(np.float64)
    b_emb = inp["b_emb"].astype(np.float64)
    b_pos = inp["b_pos"].astype(np.float64)

    W_eff = W_ih @ (W_emb @ W_pos) + W_hh                 # [4H, H]
    b_eff = W_ih @ (W_emb @ b_pos + b_emb) + b_ih + b_hh  # [4H]
    W0 = W_ih @ W_emb                                     # [4H, 2]
    b0 = W_ih @ b_emb + b_ih + b_hh                       # [4H]

    p = _GATE_PERM
    W_effT = np.ascontiguousarray(W_eff[p].T, np.float32)   # [H, 4H]
    W_hhT = np.ascontiguousarray(W_hh[p].T, np.float32)     # [H, 4H]
    W0T = np.ascontiguousarray(W0[p].T, np.float32)         # [2, 4H]
    W_posT = np.ascontiguousarray(W_pos.T, np.float32)      # [H, 2]
    beff_p = np.ascontiguousarray(b_eff[p], np.float32)[None]  # [1, 4H]
    b0_p = np.ascontiguousarray(b0[p], np.float32)[None]

    has_beff = bool(np.any(beff_p))
    has_b0 = bool(np.any(b0_p))

    h0T = np.ascontiguousarray(inp["hh"][0].T, np.float32)   # [H, B]
    c0T = np.ascontiguousarray(inp["ch"][0].T, np.float32)
    lprT = np.ascontiguousarray(inp["last_pos_rel"].T, np.float32)  # [2, B]

    nc = _get_module(has_beff, has_b0)

    in_maps = []
    for c in range(NCORES):
        cols = slice(c * BSH, (c + 1) * BSH)
        m = {
            "w_effT": W_effT,
            "w_hhT": W_hhT,
            "w0T": W0T,
            "w_posT": W_posT,
            "h0T": np.ascontiguousarray(h0T[:, cols]),
            "c0T": np.ascontiguousarray(c0T[:, cols]),
            "lprT": np.ascontiguousarray(lprT[:, cols]),
        }
        if has_beff:
            m["beff"] = beff_p
        if has_b0:
            m["b0"] = b0_p
        in_maps.append(m)

    trace = bool(int(os.environ.get("KBENCH_TRACE", "0")))
    tmpdir = os.environ.get("KBENCH_TMPDIR") or None
    res = _run_spmd(nc, in_maps, list(range(NCORES)), trace=trace, tmpdir=tmpdir)
    LAST_RESULTS = res

    traj_parts = []
    h_parts = []
    for c in range(NCORES):
        tr = np.asarray(res.results[c]["traj"])       # [SEQ, H, SUBS, 2]
        traj_parts.append(tr.transpose(0, 2, 1, 3).reshape(SEQ, BSH, 2))
        h_parts.append(np.asarray(res.results[c]["hf"]).T)  # [BSH, H]

    traj = np.concatenate(traj_parts, axis=1)
    traj = (traj + inp["b_pos"][None, None, :].astype(np.float32)).astype(np.float32)
    hF = np.concatenate(h_parts, axis=0)[None].astype(np.float32)
    return traj, hF
